# revision 6
# baseline (speedup 1.0000x reference)
"""MGCN (multi-graph GCN layer) Trainium2 kernel.

Math: with K0/K1/K2 = kernel rows interleaved (kernel[d*3+mx, u]),
  out[b] = X[b] @ K0 + bias + A0 @ (X[b] @ K1) + A1 @ (X[b] @ K2)
because the SpMM (over nodes) commutes with the per-feature projection.

Sharding: batch-parallel. Core c owns batches [c*BPC, (c+1)*BPC) and computes
the full [N, BPC*U] output slice. All cores run the identical program (SPMD)
on different data; the edge metadata is identical across cores.

Stage 1 (projection): for each 128-node tile and batch b, PE computes
  psum[128n, 2U] = xT_tile[KD=D+1, 128n].T @ [K1|K2][KD, 2U]
(the extra contraction row is ones; it carries the bias only in the K0 path)
and the result is written to HBM scratch Y1/Y2 [N, BPC*U] in bf16.

Stage 2 (SpMM + combine): per output block of 128 rows, edges of both supports
(sorted by (row//32) subgroup, padded to 128-edge tiles) are gathered with
dma_gather from Y1/Y2 (1KB rows) into G[128 edges, FC] tiles; PE accumulates
  psum[32j:32j+32, FC] += SelT[128e, 32r].T @ G[128e, FC]
where SelT holds edge vals at local-row positions (segment-sum as matmul),
then 8 more matmuls add X[b]@K0+bias, and the bank is copied out.
"""

import math
from dataclasses import dataclass, field

import numpy as np
import ml_dtypes

import concourse.bass as bass
import concourse.bacc as bacc
import concourse.mybir as mybir
from concourse.tile import TileContext, add_dep_helper

F32 = mybir.dt.float32
BF16 = mybir.dt.bfloat16
I16 = mybir.dt.int16


@dataclass
class Cfg:
    B: int = 64          # total batches
    N: int = 10000       # nodes
    D: int = 64          # input features
    U: int = 64          # units
    n_cores: int = 8
    GU: int = 8          # max gather-unit size in 128-edge tiles
                         # (dma_gather with >1024 idxs fails at runtime)
    CHUNK: int = 1024    # stage-1 node chunk (multiple of 128)
    DMA_SCRATCH: int = 16384  # SWDGE descriptor-ring scratch (bytes/partition)

    @property
    def BPC(self):       # batches per core
        return self.B // self.n_cores

    @property
    def FC(self):        # feature columns per core
        return self.BPC * self.U

    @property
    def KD(self):        # contraction dim incl. ones row
        return self.D + 1

    @property
    def NBLK(self):      # output blocks of 128 rows
        return (self.N + 127) // 128


@dataclass
class Unit:
    s: int               # support (0 -> Y1, 1 -> Y2)
    T: int               # tiles of 128 edges
    jmap: list           # per-tile subgroup (0..3)
    ioff: int            # column offset into idx_all
    soff: int            # column offset into sel_all


@dataclass
class EdgeMeta:
    units_by_block: list            # [NBLK] -> list[Unit]
    idx_all: np.ndarray             # [128, IDXW] int16
    sel_all: np.ndarray             # [128, SELW] bf16


def preprocess_edges(cfg: Cfg, supports):
    """supports: [(rows, cols, vals), (rows, cols, vals)] int/float numpy.
    Returns EdgeMeta (identical for all cores)."""
    N = cfg.N
    units_by_block = [[] for _ in range(cfg.NBLK)]
    idx_chunks, sel_chunks = [], []
    ioff = soff = 0

    sorted_sup = []
    for rows, cols, vals in supports:
        order = np.argsort(rows, kind="stable")
        sorted_sup.append((rows[order], cols[order], vals[order]))

    for blk in range(cfg.NBLK):
        for s, (r, c, v) in enumerate(sorted_sup):
            tiles = []  # (idx128 int16, sel [128, 32] f32, j)
            for j in range(4):
                lo = blk * 128 + 32 * j
                hi = min(lo + 32, N)
                if lo >= N:
                    break
                a, b = np.searchsorted(r, lo), np.searchsorted(r, hi)
                cnt = b - a
                if cnt == 0:
                    continue
                pad = (-cnt) % 128
                e_c = np.concatenate([c[a:b], np.zeros(pad, np.int64)])
                e_v = np.concatenate([v[a:b], np.zeros(pad, np.float32)])
                e_lr = np.concatenate([r[a:b] - lo, np.zeros(pad, np.int64)])
                for t in range(len(e_c) // 128):
                    sl = slice(t * 128, (t + 1) * 128)
                    sel = np.zeros((128, 32), np.float32)
                    sel[np.arange(128), e_lr[sl]] = e_v[sl]
                    tiles.append((e_c[sl].astype(np.int16), sel, j))
            for u0 in range(0, len(tiles), cfg.GU):
                ut = tiles[u0:u0 + cfg.GU]
                T = len(ut)
                flat = np.concatenate([t[0] for t in ut])          # [T*128]
                wrapped = flat.reshape(T * 8, 16).T                # [16, T*8]
                idx_chunks.append(np.tile(wrapped, (8, 1)))        # [128, T*8]
                sel_chunks.append(
                    np.concatenate([t[1] for t in ut], axis=1)     # [128, T*32]
                    .astype(ml_dtypes.bfloat16))
                units_by_block[blk].append(
                    Unit(s=s, T=T, jmap=[t[2] for t in ut], ioff=ioff, soff=soff))
                ioff += T * 8
                soff += T * 32

    idx_all = np.concatenate(idx_chunks, axis=1)
    sel_all = np.concatenate(sel_chunks, axis=1)
    return EdgeMeta(units_by_block, np.ascontiguousarray(idx_all),
                    np.ascontiguousarray(sel_all))


def prep_weights(cfg: Cfg, kernel, bias):
    """kernel [D*3, U] with rows ordered d*3+mx; bias [U]."""
    K = kernel.reshape(cfg.D, 3, cfg.U)
    kc12 = np.zeros((cfg.KD, 2 * cfg.U), np.float32)
    kc12[:cfg.D, :cfg.U] = K[:, 1]
    kc12[:cfg.D, cfg.U:] = K[:, 2]
    k0b = np.zeros((cfg.KD, cfg.U), np.float32)
    k0b[:cfg.D] = K[:, 0]
    k0b[cfg.D] = bias
    return kc12, k0b


def prep_x_core(cfg: Cfg, x, core):
    """x [B, N, D] -> xt [BPC, KD, N] f32 (transposed slice + ones row)."""
    xs = x[core * cfg.BPC:(core + 1) * cfg.BPC]          # [BPC, N, D]
    xt = np.empty((cfg.BPC, cfg.KD, cfg.N), np.float32)
    xt[:, :cfg.D, :] = xs.transpose(0, 2, 1)
    xt[:, cfg.D, :] = 1.0
    return np.ascontiguousarray(xt)


def build_nc(cfg: Cfg, meta: EdgeMeta):
    nc = bacc.Bacc("TRN2", num_devices=cfg.n_cores,
                   dynamic_dma_scratch_size=cfg.DMA_SCRATCH)
    KD, FC, U, N, BPC = cfg.KD, cfg.FC, cfg.U, cfg.N, cfg.BPC

    xt_t = nc.dram_tensor("xt", [BPC, KD, N], F32, kind="ExternalInput")
    kc12_t = nc.dram_tensor("kc12", [KD, 2 * U], F32, kind="ExternalInput")
    k0b_t = nc.dram_tensor("k0b", [KD, U], F32, kind="ExternalInput")
    idx_t = nc.dram_tensor("idx16", list(meta.idx_all.shape), I16,
                           kind="ExternalInput")
    sel_t = nc.dram_tensor("sel", list(meta.sel_all.shape), BF16,
                           kind="ExternalInput")
    y_t = [nc.dram_tensor(f"y{s}", [N, FC], BF16, kind="Internal")
           for s in (0, 1)]
    out_t = nc.dram_tensor("out", [BPC, N, U], F32, kind="ExternalOutput")

    with TileContext(nc) as tc:
        with tc.tile_pool(name="kpool", bufs=1) as kpool:
            kc_sb = kpool.tile([KD, 2 * U], F32, tag="kc")
            nc.sync.dma_start(kc_sb[:, :], kc12_t.ap()[:, :])
            k0b_sb = kpool.tile([KD, U], F32, tag="k0b")
            nc.sync.dma_start(k0b_sb[:, :], k0b_t.ap()[:, :])

            # ---------------- Stage 1: projection ----------------
            with tc.tile_pool(name="xc", bufs=2) as xcpool, \
                 tc.tile_pool(name="st1", bufs=3) as stpool, \
                 tc.tile_pool(name="ps1", bufs=4, space="PSUM") as ps1pool:
                for c0 in range(0, N, cfg.CHUNK):
                    cw = min(cfg.CHUNK, N - c0)
                    xc = xcpool.tile([KD, BPC, cw], F32, tag="xc")
                    for b in range(BPC):
                        nc.sync.dma_start(xc[:, b, :], xt_t.ap()[b, :, c0:c0 + cw])
                    for t0 in range(0, cw, 128):
                        nn = min(128, cw - t0)
                        st = stpool.tile([128, 2, FC], BF16, tag="st")
                        for b in range(BPC):
                            pp = ps1pool.tile([128, 2 * U], F32, tag="pp")
                            nc.tensor.matmul(pp[:nn, :], xc[:, b, t0:t0 + nn],
                                             kc_sb[:, :], start=True, stop=True)
                            nc.any.tensor_copy(
                                st[:nn, :, b * U:(b + 1) * U],
                                pp[:nn, :].rearrange("p (s u) -> p s u", s=2))
                        n0 = c0 + t0
                        nc.sync.dma_start(y_t[0].ap()[n0:n0 + nn, :], st[:nn, 0, :])
                        nc.sync.dma_start(y_t[1].ap()[n0:n0 + nn, :], st[:nn, 1, :])

            tc.strict_bb_all_engine_barrier()

            # ---------------- Stage 2: SpMM + combine ----------------
            with tc.tile_pool(name="gp", bufs=3) as gpool, \
                 tc.tile_pool(name="ip", bufs=3) as ipool, \
                 tc.tile_pool(name="sp", bufs=3) as spool, \
                 tc.tile_pool(name="xb", bufs=2) as xbpool, \
                 tc.tile_pool(name="op", bufs=2) as opool, \
                 tc.tile_pool(name="ps2", bufs=2, space="PSUM") as ps2pool:
                for blk in range(cfg.NBLK):
                    n0 = blk * 128
                    nn = min(128, N - n0)
                    groups = [j for j in range(4) if 32 * j < nn]
                    ps = ps2pool.tile([128, FC], F32, tag="ps")

                    # Load the xt slice for the X@K0+bias term first.
                    xtt = xbpool.tile([KD, BPC, 128], F32, tag="xtt")
                    for b in range(BPC):
                        nc.sync.dma_start(xtt[:, b, :nn],
                                          xt_t.ap()[b, :, n0:n0 + nn])

                    # The PSUM start/stop accumulation-group flags are
                    # per-partition-region, so each 32-row col-group needs its
                    # own start (first matmul) and stop (last matmul). Collect
                    # all matmuls of the block first, then emit them chained
                    # with scheduler-only deps: the Tile scheduler would
                    # otherwise reorder writes to disjoint PSUM slices, which
                    # breaks the group semantics (PE is in-order, so the deps
                    # cost nothing).
                    specs = []  # (out, lhsT, rhs, j, tile_position)
                    for j in groups:
                        rj = min(32, nn - 32 * j)
                        for b in range(BPC):
                            specs.append((ps[32 * j:32 * j + rj,
                                             b * U:(b + 1) * U],
                                          xtt[:, b, 32 * j:32 * j + rj],
                                          k0b_sb[:, :], j))
                    for u in meta.units_by_block[blk]:
                        it = ipool.tile([128, u.T * 8], I16, tag="idx")
                        nc.sync.dma_start(it[:, :],
                                          idx_t.ap()[:, u.ioff:u.ioff + u.T * 8])
                        st = spool.tile([128, u.T * 32], BF16, tag="sel")
                        nc.sync.dma_start(st[:, :],
                                          sel_t.ap()[:, u.soff:u.soff + u.T * 32])
                        gt = gpool.tile([128, u.T, FC], BF16, tag="g")
                        nc.gpsimd.dma_gather(
                            gt[:, :, :], y_t[u.s].ap()[:, :], it[:, :],
                            num_idxs=u.T * 128, num_idxs_reg=u.T * 128,
                            elem_size=FC)
                        for t in range(u.T):
                            specs.append((ps[32 * u.jmap[t]:32 * (u.jmap[t] + 1), :],
                                          st[:, t * 32:(t + 1) * 32],
                                          gt[:, t, :], u.jmap[t]))
                    first_of_group = {}
                    last_of_group = {}
                    for i, sp in enumerate(specs):
                        first_of_group.setdefault(sp[3], i)
                        last_of_group[sp[3]] = i
                    prev_mm = None
                    for i, (out_ap, lhsT, rhs, j) in enumerate(specs):
                        # skip_group_check: the CoreSim group-check shadow
                        # mis-addresses partition-sliced PSUM outputs (false
                        # "pending group" collisions); the per-tensor pending-
                        # zero data path models start/stop correctly.
                        mm = nc.tensor.matmul(
                            out_ap, lhsT, rhs,
                            start=(first_of_group[j] == i),
                            stop=(last_of_group[j] == i),
                            tile_position=(0, 32 * j),
                            skip_group_check=True)
                        if prev_mm is not None:
                            add_dep_helper(mm.ins, prev_mm.ins, sync=False,
                                           reason="psum accumulation order")
                        prev_mm = mm

                    ot = opool.tile([128, FC], F32, tag="ot")
                    nc.any.tensor_copy(ot[:nn, :], ps[:nn, :])
                    for b in range(BPC):
                        nc.scalar.dma_start(out_t.ap()[b, n0:n0 + nn, :],
                                            ot[:nn, b * U:(b + 1) * U])
    return nc


def make_in_maps(cfg: Cfg, inputs, meta: EdgeMeta, kc12, k0b):
    x = np.asarray(inputs["x"], np.float32)
    in_maps = []
    for c in range(cfg.n_cores):
        in_maps.append({
            "xt": prep_x_core(cfg, x, c),
            "kc12": kc12,
            "k0b": k0b,
            "idx16": meta.idx_all,
            "sel": meta.sel_all,
        })
    return in_maps


def run(cfg: Cfg, inputs, trace=False, **spmd_kwargs):
    """inputs: dict as from setup_inputs (numpy). Returns out [B, N, U] f32
    (and the BassKernelResults)."""
    supports = [(np.asarray(inputs["sup0_rows"]), np.asarray(inputs["sup0_cols"]),
                 np.asarray(inputs["sup0_vals"], np.float32)),
                (np.asarray(inputs["sup1_rows"]), np.asarray(inputs["sup1_cols"]),
                 np.asarray(inputs["sup1_vals"], np.float32))]
    meta = preprocess_edges(cfg, supports)
    kc12, k0b = prep_weights(cfg, np.asarray(inputs["kernel"], np.float32),
                             np.asarray(inputs["bias"], np.float32))
    nc = build_nc(cfg, meta)
    nc.compile()
    in_maps = make_in_maps(cfg, inputs, meta, kc12, k0b)

    from concourse.bass_utils import run_bass_kernel_spmd
    res = run_bass_kernel_spmd(nc, in_maps, core_ids=list(range(cfg.n_cores)),
                               trace=trace, **spmd_kwargs)
    out = np.concatenate([res.results[c]["out"] for c in range(cfg.n_cores)],
                         axis=0)
    return out, res


def kernel(**inputs) -> np.ndarray:
    """Full MGCN layer: takes the unsharded inputs of reference.setup_inputs()
    and returns the full [B, N, UNITS] float32 output."""
    out, _ = run(Cfg(), inputs, trace=False)
    return np.asarray(out, np.float32)


# revision 10
# speedup vs baseline: 1.1242x; 1.1242x over previous
"""MGCN (multi-graph GCN layer) Trainium2 kernel.

Math: with K0/K1/K2 = kernel rows interleaved (kernel[d*3+mx, u]),
  out[b] = X[b] @ K0 + bias + A0 @ (X[b] @ K1) + A1 @ (X[b] @ K2)
because the SpMM (over nodes) commutes with the per-feature projection.

Sharding: batch-parallel. Core c owns batches [c*BPC, (c+1)*BPC) and computes
the full [N, BPC*U] output slice. All cores run the identical program (SPMD)
on different data; the edge metadata is identical across cores.

Stage 1 (projection): for each 128-node tile and batch b, PE computes
  psum[128n, 2U] = xT_tile[KD=D+1, 128n].T @ [K1|K2][KD, 2U]
(the extra contraction row is ones; it carries the bias only in the K0 path)
and the result is written to HBM scratch Y1/Y2 [N, BPC*U] in bf16.

Stage 2 (SpMM + combine): per output block of 128 rows, edges of both supports
(sorted by (row//32) subgroup, padded to 128-edge tiles) are gathered with
dma_gather from Y1/Y2 (1KB rows) into G[128 edges, FC] tiles; PE accumulates
  psum[32j:32j+32, FC] += SelT[128e, 32r].T @ G[128e, FC]
where SelT holds edge vals at local-row positions (segment-sum as matmul),
then 8 more matmuls add X[b]@K0+bias, and the bank is copied out.
"""

import math
from dataclasses import dataclass, field

import numpy as np
import ml_dtypes

import concourse.bass as bass
import concourse.bacc as bacc
import concourse.mybir as mybir
from concourse.tile import TileContext, add_dep_helper

F32 = mybir.dt.float32
BF16 = mybir.dt.bfloat16
I16 = mybir.dt.int16


@dataclass
class Cfg:
    B: int = 64          # total batches
    N: int = 10000       # nodes
    D: int = 64          # input features
    U: int = 64          # units
    n_cores: int = 8
    GU: int = 8          # max gather-unit size in 128-edge tiles
                         # (dma_gather with >1024 idxs fails at runtime)
    CHUNK: int = 1024    # stage-1 node chunk (multiple of 128)
    DMA_SCRATCH: int = 16384  # SWDGE descriptor-ring scratch (bytes/partition)
    NQ: int = 4          # SWDGE queues; gathers round-robin across them

    @property
    def BPC(self):       # batches per core
        return self.B // self.n_cores

    @property
    def FC(self):        # feature columns per core
        return self.BPC * self.U

    @property
    def KD(self):        # contraction dim incl. ones row
        return self.D + 1

    @property
    def NBLK(self):      # output blocks of 128 rows
        return (self.N + 127) // 128


@dataclass
class Unit:
    s: int               # support (0 -> Y1, 1 -> Y2)
    T: int               # tiles of 128 edges
    jmap: list           # per-tile subgroup (0..3)
    ioff: int            # column offset into idx_all
    soff: int            # column offset into sel_all


@dataclass
class EdgeMeta:
    units_by_block: list            # [NBLK] -> list[Unit]
    idx_all: np.ndarray             # [128, IDXW] int16
    sel_all: np.ndarray             # [128, SELW] bf16


def preprocess_edges(cfg: Cfg, supports):
    """supports: [(rows, cols, vals), (rows, cols, vals)] int/float numpy.
    Returns EdgeMeta (identical for all cores)."""
    N = cfg.N
    units_by_block = [[] for _ in range(cfg.NBLK)]
    idx_chunks, sel_chunks = [], []
    ioff = soff = 0

    sorted_sup = []
    for rows, cols, vals in supports:
        order = np.argsort(rows, kind="stable")
        sorted_sup.append((rows[order], cols[order], vals[order]))

    for blk in range(cfg.NBLK):
        for s, (r, c, v) in enumerate(sorted_sup):
            tiles = []  # (idx128 int16, sel [128, 32] f32, j)
            for j in range(4):
                lo = blk * 128 + 32 * j
                hi = min(lo + 32, N)
                if lo >= N:
                    break
                a, b = np.searchsorted(r, lo), np.searchsorted(r, hi)
                cnt = b - a
                if cnt == 0:
                    continue
                pad = (-cnt) % 128
                e_c = np.concatenate([c[a:b], np.zeros(pad, np.int64)])
                e_v = np.concatenate([v[a:b], np.zeros(pad, np.float32)])
                e_lr = np.concatenate([r[a:b] - lo, np.zeros(pad, np.int64)])
                for t in range(len(e_c) // 128):
                    sl = slice(t * 128, (t + 1) * 128)
                    sel = np.zeros((128, 32), np.float32)
                    sel[np.arange(128), e_lr[sl]] = e_v[sl]
                    tiles.append((e_c[sl].astype(np.int16), sel, j))
            for u0 in range(0, len(tiles), cfg.GU):
                ut = tiles[u0:u0 + cfg.GU]
                T = len(ut)
                flat = np.concatenate([t[0] for t in ut])          # [T*128]
                wrapped = flat.reshape(T * 8, 16).T                # [16, T*8]
                idx_chunks.append(np.tile(wrapped, (8, 1)))        # [128, T*8]
                sel_chunks.append(
                    np.concatenate([t[1] for t in ut], axis=1)     # [128, T*32]
                    .astype(ml_dtypes.bfloat16))
                units_by_block[blk].append(
                    Unit(s=s, T=T, jmap=[t[2] for t in ut], ioff=ioff, soff=soff))
                ioff += T * 8
                soff += T * 32

    idx_all = np.concatenate(idx_chunks, axis=1)
    sel_all = np.concatenate(sel_chunks, axis=1)
    return EdgeMeta(units_by_block, np.ascontiguousarray(idx_all),
                    np.ascontiguousarray(sel_all))


def prep_weights(cfg: Cfg, kernel, bias):
    """kernel [D*3, U] with rows ordered d*3+mx; bias [U]."""
    K = kernel.reshape(cfg.D, 3, cfg.U)
    kc12 = np.zeros((cfg.KD, 2 * cfg.U), np.float32)
    kc12[:cfg.D, :cfg.U] = K[:, 1]
    kc12[:cfg.D, cfg.U:] = K[:, 2]
    k0b = np.zeros((cfg.KD, cfg.U), np.float32)
    k0b[:cfg.D] = K[:, 0]
    k0b[cfg.D] = bias
    return kc12, k0b


def prep_x_core(cfg: Cfg, x, core):
    """x [B, N, D] -> xt [BPC, KD, N] f32 (transposed slice + ones row)."""
    xs = x[core * cfg.BPC:(core + 1) * cfg.BPC]          # [BPC, N, D]
    xt = np.empty((cfg.BPC, cfg.KD, cfg.N), np.float32)
    xt[:, :cfg.D, :] = xs.transpose(0, 2, 1)
    xt[:, cfg.D, :] = 1.0
    return np.ascontiguousarray(xt)


def build_nc(cfg: Cfg, meta: EdgeMeta):
    nc = bacc.Bacc("TRN2", num_devices=cfg.n_cores,
                   dynamic_dma_scratch_size=cfg.DMA_SCRATCH,
                   num_swdge_queues=cfg.NQ)
    KD, FC, U, N, BPC = cfg.KD, cfg.FC, cfg.U, cfg.N, cfg.BPC

    xt_t = nc.dram_tensor("xt", [BPC, KD, N], F32, kind="ExternalInput")
    kc12_t = nc.dram_tensor("kc12", [KD, 2 * U], F32, kind="ExternalInput")
    k0b_t = nc.dram_tensor("k0b", [KD, U], F32, kind="ExternalInput")
    idx_t = nc.dram_tensor("idx16", list(meta.idx_all.shape), I16,
                           kind="ExternalInput")
    sel_t = nc.dram_tensor("sel", list(meta.sel_all.shape), BF16,
                           kind="ExternalInput")
    y_t = [nc.dram_tensor(f"y{s}", [N, FC], BF16, kind="Internal")
           for s in (0, 1)]
    out_t = nc.dram_tensor("out", [BPC, N, U], F32, kind="ExternalOutput")

    with TileContext(nc) as tc:
        with tc.tile_pool(name="kpool", bufs=1) as kpool:
            kc_sb = kpool.tile([KD, 2 * U], F32, tag="kc")
            nc.sync.dma_start(kc_sb[:, :], kc12_t.ap()[:, :])
            k0b_sb = kpool.tile([KD, U], F32, tag="k0b")
            nc.sync.dma_start(k0b_sb[:, :], k0b_t.ap()[:, :])

            # ---------------- Stage 1: projection ----------------
            with tc.tile_pool(name="xc", bufs=2) as xcpool, \
                 tc.tile_pool(name="st1", bufs=3) as stpool, \
                 tc.tile_pool(name="ps1", bufs=4, space="PSUM") as ps1pool:
                for c0 in range(0, N, cfg.CHUNK):
                    cw = min(cfg.CHUNK, N - c0)
                    xc = xcpool.tile([KD, BPC, cw], F32, tag="xc")
                    for b in range(BPC):
                        nc.sync.dma_start(xc[:, b, :], xt_t.ap()[b, :, c0:c0 + cw])
                    for t0 in range(0, cw, 128):
                        nn = min(128, cw - t0)
                        st = stpool.tile([128, 2, FC], BF16, tag="st")
                        for b in range(BPC):
                            pp = ps1pool.tile([128, 2 * U], F32, tag="pp")
                            nc.tensor.matmul(pp[:nn, :], xc[:, b, t0:t0 + nn],
                                             kc_sb[:, :], start=True, stop=True)
                            nc.any.tensor_copy(
                                st[:nn, :, b * U:(b + 1) * U],
                                pp[:nn, :].rearrange("p (s u) -> p s u", s=2))
                        n0 = c0 + t0
                        nc.sync.dma_start(y_t[0].ap()[n0:n0 + nn, :], st[:nn, 0, :])
                        nc.sync.dma_start(y_t[1].ap()[n0:n0 + nn, :], st[:nn, 1, :])

            tc.strict_bb_all_engine_barrier()

            # ---------------- Stage 2: SpMM + combine ----------------
            with tc.tile_pool(name="gp", bufs=3) as gpool, \
                 tc.tile_pool(name="ip", bufs=3) as ipool, \
                 tc.tile_pool(name="sp", bufs=3) as spool, \
                 tc.tile_pool(name="xb", bufs=2) as xbpool, \
                 tc.tile_pool(name="op", bufs=2) as opool, \
                 tc.tile_pool(name="ps2", bufs=2, space="PSUM") as ps2pool:
                gq = 0
                for blk in range(cfg.NBLK):
                    n0 = blk * 128
                    nn = min(128, N - n0)
                    groups = [j for j in range(4) if 32 * j < nn]
                    ps = ps2pool.tile([128, FC], F32, tag="ps")

                    # Load the xt slice for the X@K0+bias term first.
                    xtt = xbpool.tile([KD, BPC, 128], F32, tag="xtt")
                    for b in range(BPC):
                        nc.sync.dma_start(xtt[:, b, :nn],
                                          xt_t.ap()[b, :, n0:n0 + nn])

                    # The PSUM start/stop accumulation-group flags are
                    # per-partition-region, so each 32-row col-group needs its
                    # own start (first matmul) and stop (last matmul). Collect
                    # all matmuls of the block first, then emit them chained
                    # with scheduler-only deps: the Tile scheduler would
                    # otherwise reorder writes to disjoint PSUM slices, which
                    # breaks the group semantics (PE is in-order, so the deps
                    # cost nothing).
                    specs = []  # (out, lhsT, rhs, j, tile_position)
                    for j in groups:
                        rj = min(32, nn - 32 * j)
                        for b in range(BPC):
                            specs.append((ps[32 * j:32 * j + rj,
                                             b * U:(b + 1) * U],
                                          xtt[:, b, 32 * j:32 * j + rj],
                                          k0b_sb[:, :], j))
                    for u in meta.units_by_block[blk]:
                        it = ipool.tile([128, u.T * 8], I16, tag="idx")
                        nc.sync.dma_start(it[:, :],
                                          idx_t.ap()[:, u.ioff:u.ioff + u.T * 8])
                        st = spool.tile([128, u.T * 32], BF16, tag="sel")
                        nc.sync.dma_start(st[:, :],
                                          sel_t.ap()[:, u.soff:u.soff + u.T * 32])
                        gt = gpool.tile([128, u.T, FC], BF16, tag="g")
                        nc.gpsimd.dma_gather(
                            gt[:, :, :], y_t[u.s].ap()[:, :], it[:, :],
                            num_idxs=u.T * 128, num_idxs_reg=u.T * 128,
                            elem_size=FC, queue_num=gq % cfg.NQ)
                        gq += 1
                        for t in range(u.T):
                            specs.append((ps[32 * u.jmap[t]:32 * (u.jmap[t] + 1), :],
                                          st[:, t * 32:(t + 1) * 32],
                                          gt[:, t, :], u.jmap[t]))
                    first_of_group = {}
                    last_of_group = {}
                    for i, sp in enumerate(specs):
                        first_of_group.setdefault(sp[3], i)
                        last_of_group[sp[3]] = i
                    prev_mm = None
                    for i, (out_ap, lhsT, rhs, j) in enumerate(specs):
                        # skip_group_check: the CoreSim group-check shadow
                        # mis-addresses partition-sliced PSUM outputs (false
                        # "pending group" collisions); the per-tensor pending-
                        # zero data path models start/stop correctly.
                        mm = nc.tensor.matmul(
                            out_ap, lhsT, rhs,
                            start=(first_of_group[j] == i),
                            stop=(last_of_group[j] == i),
                            tile_position=(0, 32 * j),
                            skip_group_check=True)
                        if prev_mm is not None:
                            add_dep_helper(mm.ins, prev_mm.ins, sync=False,
                                           reason="psum accumulation order")
                        prev_mm = mm

                    ot = opool.tile([128, FC], F32, tag="ot")
                    nc.any.tensor_copy(ot[:nn, :], ps[:nn, :])
                    for b in range(BPC):
                        nc.scalar.dma_start(out_t.ap()[b, n0:n0 + nn, :],
                                            ot[:nn, b * U:(b + 1) * U])
    return nc


def make_in_maps(cfg: Cfg, inputs, meta: EdgeMeta, kc12, k0b):
    x = np.asarray(inputs["x"], np.float32)
    in_maps = []
    for c in range(cfg.n_cores):
        in_maps.append({
            "xt": prep_x_core(cfg, x, c),
            "kc12": kc12,
            "k0b": k0b,
            "idx16": meta.idx_all,
            "sel": meta.sel_all,
        })
    return in_maps


def run(cfg: Cfg, inputs, trace=False, **spmd_kwargs):
    """inputs: dict as from setup_inputs (numpy). Returns out [B, N, U] f32
    (and the BassKernelResults)."""
    supports = [(np.asarray(inputs["sup0_rows"]), np.asarray(inputs["sup0_cols"]),
                 np.asarray(inputs["sup0_vals"], np.float32)),
                (np.asarray(inputs["sup1_rows"]), np.asarray(inputs["sup1_cols"]),
                 np.asarray(inputs["sup1_vals"], np.float32))]
    meta = preprocess_edges(cfg, supports)
    kc12, k0b = prep_weights(cfg, np.asarray(inputs["kernel"], np.float32),
                             np.asarray(inputs["bias"], np.float32))
    nc = build_nc(cfg, meta)
    nc.compile()
    in_maps = make_in_maps(cfg, inputs, meta, kc12, k0b)

    from concourse.bass_utils import run_bass_kernel_spmd
    res = run_bass_kernel_spmd(nc, in_maps, core_ids=list(range(cfg.n_cores)),
                               trace=trace, **spmd_kwargs)
    out = np.concatenate([res.results[c]["out"] for c in range(cfg.n_cores)],
                         axis=0)
    return out, res


def kernel(**inputs) -> np.ndarray:
    """Full MGCN layer: takes the unsharded inputs of reference.setup_inputs()
    and returns the full [B, N, UNITS] float32 output."""
    out, _ = run(Cfg(), inputs, trace=False)
    return np.asarray(out, np.float32)


# revision 13
# speedup vs baseline: 1.6317x; 1.4514x over previous
"""MGCN (multi-graph GCN layer) Trainium2 kernel.

Math: with K0/K1/K2 = kernel rows de-interleaved (kernel[d*3+mx, u]),
  out[b] = X[b] @ K0 + bias + A0 @ (X[b] @ K1) + A1 @ (X[b] @ K2)
because the SpMM (over nodes) commutes with the per-feature projection.

Sharding: node-parallel for the SpMM. Core c owns output rows
[c*1250, (c+1)*1250) for ALL 64 batches. Every core redundantly computes the
full projections Y1 = X@K1, Y2 = X@K2 (cheap in bf16 on the PE) and writes
them row-interleaved into a local HBM scratch Y12[2n+s] = Ys[n] of shape
[2N, B*U] bf16, so the SpMM gather needs no cross-core traffic.

Stage 2: per output block of 128 rows, the edges of both supports (grouped by
32-row subgroup, sorted, padded to 128-edge tiles; padded to a uniform tile
count so all 8 cores run one identical SPMD program) are gathered with
dma_gather as full 8KB bf16 rows of Y12 (idx = 2*col + support), and the PE
accumulates segment sums via selector matmuls
  psum_f[32j:32j+32, :] += SelT[128e, 32r].T @ G[128e, f*512:(f+1)*512]
into 8 chunk-PSUM banks (one per group of 8 batches). The X@K0+bias term is
added by small per-(j, batch) matmuls from a per-core xt slice, then each
bank is copied out and written strided into the [B, N, U] output.

The single dma_gather descriptor per edge moves 8KB, which keeps the GpSimd
(SWDGE descriptor generation) cost ~8x below the HBM/DMA time — the kernel is
HBM-bound on the irreducible gather traffic.
"""

import math
from dataclasses import dataclass, field

import numpy as np
import ml_dtypes

import concourse.bass as bass
import concourse.bacc as bacc
import concourse.mybir as mybir
from concourse.tile import TileContext, add_dep_helper

F32 = mybir.dt.float32
BF16 = mybir.dt.bfloat16
I16 = mybir.dt.int16


@dataclass
class Cfg:
    B: int = 64          # total batches
    N: int = 10000       # nodes
    D: int = 64          # input features
    U: int = 64          # units
    n_cores: int = 8
    GU: int = 4          # gather-unit size in 128-edge tiles (elem = 8KB)
    CHUNK: int = 256     # stage-1 node chunk (multiple of 128)
    DMA_SCRATCH: int = 16384
    NQ: int = 4          # SWDGE queues; gathers round-robin across them

    @property
    def F(self):         # full feature width B*U
        return self.B * self.U

    @property
    def NPC(self):       # nodes (output rows) per core
        return self.N // self.n_cores

    @property
    def KD(self):        # contraction dim incl. ones row
        return self.D + 1

    @property
    def NT(self):        # stage-1 node tiles of 128 (full projection)
        return (self.N + 127) // 128

    @property
    def NBLK(self):      # per-core output blocks of 128 rows
        return (self.NPC + 127) // 128

    @property
    def NCHUNK(self):    # 512-col feature chunks
        return self.F // 512


@dataclass
class EdgeMeta:
    tiles_per_group: int            # uniform T for every (blk, j)
    jmap: list                      # per-block flat tile -> j (same all blocks)
    idx_shape: tuple
    sel_shape: tuple


def preprocess_edges(cfg: Cfg, supports):
    """Build per-core idx/sel arrays with a uniform SPMD structure.

    Returns (meta, idx_by_core [n_cores, 128, W_i], sel_by_core).
    Edge (r, c, v) of support s gathers Y12 row 2c+s; it lands in core
    r // NPC, block (r % NPC) // 128, subgroup ((r % NPC) % 128) // 32.
    """
    N, NPC = cfg.N, cfg.NPC
    n_groups_rows = []  # per (core, blk, j): (idx_list, val, lr)
    groups = {}
    for s, (rows, cols, vals) in enumerate(supports):
        rows = np.asarray(rows)
        cols = np.asarray(cols)
        vals = np.asarray(vals, np.float32)
        order = np.argsort(rows, kind="stable")
        r, c, v = rows[order], cols[order], vals[order]
        core = r // NPC
        rr = r % NPC
        blk = rr // 128
        j = (rr % 128) // 32
        lr = rr % 32
        gidx = 2 * c + s
        key = np.stack([core, blk, j])
        for cc in range(cfg.n_cores):
            m0 = core == cc
            for bb in range(cfg.NBLK):
                m1 = m0 & (blk == bb)
                for jj in range(4):
                    m = m1 & (j == jj)
                    if not m.any():
                        continue
                    g = groups.setdefault((cc, bb, jj), [[], [], []])
                    g[0].append(gidx[m])
                    g[1].append(v[m])
                    g[2].append(lr[m])

    # uniform tile count per (blk, j)
    maxlen = 0
    for g in groups.values():
        n = sum(len(a) for a in g[0])
        maxlen = max(maxlen, n)
    T = (maxlen + 127) // 128
    # round T up to a multiple of GU so every unit is full-size
    T = ((T + cfg.GU - 1) // cfg.GU) * cfg.GU

    idx_by_core, sel_by_core = [], []
    for cc in range(cfg.n_cores):
        idx_cols, sel_cols = [], []
        for bb in range(cfg.NBLK):
            for jj in range(4):
                g = groups.get((cc, bb, jj))
                if g is None:
                    gi = np.zeros(0, np.int64)
                    gv = np.zeros(0, np.float32)
                    gl = np.zeros(0, np.int64)
                else:
                    gi = np.concatenate(g[0])
                    gv = np.concatenate(g[1])
                    gl = np.concatenate(g[2])
                pad = T * 128 - len(gi)
                gi = np.concatenate([gi, np.zeros(pad, np.int64)])
                gv = np.concatenate([gv, np.zeros(pad, np.float32)])
                gl = np.concatenate([gl, np.zeros(pad, np.int64)])
                # idx wrap: index i -> [i % 16, i // 16], replicated x8
                wrapped = gi.astype(np.int16).reshape(T * 8, 16).T
                idx_cols.append(np.tile(wrapped, (8, 1)))
                sel = np.zeros((128, T, 32), np.float32)
                lane = np.arange(T * 128) % 128
                tt = np.arange(T * 128) // 128
                sel[lane, tt, gl] = gv
                sel_cols.append(sel.reshape(128, T * 32)
                                .astype(ml_dtypes.bfloat16))
        idx_by_core.append(np.ascontiguousarray(np.concatenate(idx_cols, axis=1)))
        sel_by_core.append(np.ascontiguousarray(np.concatenate(sel_cols, axis=1)))

    jmap = []
    for jj in range(4):
        jmap += [jj] * T
    meta = EdgeMeta(tiles_per_group=T, jmap=jmap,
                    idx_shape=idx_by_core[0].shape,
                    sel_shape=sel_by_core[0].shape)
    return meta, idx_by_core, sel_by_core


def prep_weights(cfg: Cfg, kernel, bias):
    K = kernel.reshape(cfg.D, 3, cfg.U)
    kc12 = np.zeros((cfg.KD, 2 * cfg.U), np.float32)
    kc12[:cfg.D, :cfg.U] = K[:, 1]
    kc12[:cfg.D, cfg.U:] = K[:, 2]
    k0b = np.zeros((cfg.KD, cfg.U), np.float32)
    k0b[:cfg.D] = K[:, 0]
    k0b[cfg.D] = bias
    return (kc12.astype(ml_dtypes.bfloat16), k0b.astype(ml_dtypes.bfloat16))


def prep_x(cfg: Cfg, x):
    """x [B, N, D] f32 -> xt_full [KD, B, N] bf16 (d-major, ones row)."""
    xt = np.empty((cfg.KD, cfg.B, cfg.N), np.float32)
    xt[:cfg.D] = x.transpose(2, 0, 1)
    xt[cfg.D] = 1.0
    return np.ascontiguousarray(xt.astype(ml_dtypes.bfloat16))


def prep_x_core(cfg: Cfg, xt_full, core):
    """xt_own [KD, B, NPC] bf16 slice for the X@K0+bias term."""
    sl = xt_full[:, :, core * cfg.NPC:(core + 1) * cfg.NPC]
    return np.ascontiguousarray(sl)


def build_nc(cfg: Cfg, meta: EdgeMeta):
    nc = bacc.Bacc("TRN2", num_devices=cfg.n_cores,
                   dynamic_dma_scratch_size=cfg.DMA_SCRATCH,
                   num_swdge_queues=cfg.NQ)
    KD, F, U, N, B = cfg.KD, cfg.F, cfg.U, cfg.N, cfg.B
    NPC, T = cfg.NPC, meta.tiles_per_group

    xt_t = nc.dram_tensor("xt", [KD, B, N], BF16, kind="ExternalInput")
    xo_t = nc.dram_tensor("xo", [KD, B, NPC], BF16, kind="ExternalInput")
    kc12_t = nc.dram_tensor("kc12", [KD, 2 * U], BF16, kind="ExternalInput")
    k0b_t = nc.dram_tensor("k0b", [KD, U], BF16, kind="ExternalInput")
    idx_t = nc.dram_tensor("idx16", list(meta.idx_shape), I16,
                           kind="ExternalInput")
    sel_t = nc.dram_tensor("sel", list(meta.sel_shape), BF16,
                           kind="ExternalInput")
    y12_t = nc.dram_tensor("y12", [2 * N, F], BF16, kind="Internal")
    out_t = nc.dram_tensor("out", [B, NPC, U], F32, kind="ExternalOutput")

    with TileContext(nc) as tc:
        with tc.tile_pool(name="kpool", bufs=1) as kpool:
            kc_sb = kpool.tile([KD, 2 * U], BF16, tag="kc")
            nc.sync.dma_start(kc_sb[:, :], kc12_t.ap()[:, :])
            k0b_sb = kpool.tile([KD, U], BF16, tag="k0b")
            nc.sync.dma_start(k0b_sb[:, :], k0b_t.ap()[:, :])

            # ---- Stage 1: full projection Y12[2n+s] = (X @ K_{s+1})[n] ----
            with tc.tile_pool(name="xc", bufs=2) as xcpool, \
                 tc.tile_pool(name="st1", bufs=3) as stpool, \
                 tc.tile_pool(name="ps1", bufs=4, space="PSUM") as ps1pool:
                for c0 in range(0, N, cfg.CHUNK):
                    cw = min(cfg.CHUNK, N - c0)
                    xc = xcpool.tile([KD, B, cw], BF16, tag="xc")
                    nc.sync.dma_start(xc[:, :, :], xt_t.ap()[:, :, c0:c0 + cw])
                    for t0 in range(0, cw, 128):
                        nn = min(128, cw - t0)
                        st = stpool.tile([128, 2, F], BF16, tag="st")
                        for b8 in range(B // 8):
                            pp = ps1pool.tile([128, 8, 2 * U], F32, tag="pp")
                            for b2 in range(8):
                                b = b8 * 8 + b2
                                # the tile spans 2 PSUM banks; start clears
                                # one 2KB bank region, so restart per bank
                                nc.tensor.matmul(pp[:nn, b2, :],
                                                 xc[:, b, t0:t0 + nn],
                                                 kc_sb[:, :],
                                                 start=(b2 % 4 == 0),
                                                 stop=(b2 % 4 == 3),
                                                 skip_group_check=True)
                            # pp layout [n, b2, (s u)] -> st [n, s, (b2 u)]
                            nc.any.tensor_copy(
                                st[:nn, :, b8 * 512:b8 * 512 + 512]
                                .rearrange("p s (b2 u) -> p b2 s u", b2=8),
                                pp[:nn, :, :].rearrange(
                                    "p b2 (s u) -> p b2 s u", s=2))
                        n0 = c0 + t0
                        y12v = y12_t.ap().rearrange("(n s) f -> n s f", s=2)
                        nc.sync.dma_start(y12v[n0:n0 + nn, 0, :], st[:nn, 0, :])
                        nc.sync.dma_start(y12v[n0:n0 + nn, 1, :], st[:nn, 1, :])

            tc.strict_bb_all_engine_barrier()

            # ---- Stage 2: SpMM + X@K0 + bias, per 128-row block ----
            with tc.tile_pool(name="gp", bufs=3) as gpool, \
                 tc.tile_pool(name="ip", bufs=4) as ipool, \
                 tc.tile_pool(name="sp", bufs=4) as spool, \
                 tc.tile_pool(name="xb", bufs=2) as xbpool, \
                 tc.tile_pool(name="op", bufs=2) as opool, \
                 tc.tile_pool(name="ps2", bufs=1, space="PSUM") as ps2pool:
                gq = 0
                for blk in range(cfg.NBLK):
                    n0 = blk * 128
                    nn = min(128, NPC - n0)
                    groups = [j for j in range(4) if 32 * j < nn]
                    pss = [ps2pool.tile([128, 512], F32, tag=f"ps{f}",
                                        name=f"ps{f}")
                           for f in range(cfg.NCHUNK)]

                    xtt = xbpool.tile([KD, B, 128], BF16, tag="xtt")
                    nc.sync.dma_start(xtt[:, :, :nn],
                                      xo_t.ap()[:, :, n0:n0 + nn])

                    # (out, lhsT, rhs, chunk, j)
                    specs = []
                    for j in groups:
                        rj = min(32, nn - 32 * j)
                        for b in range(B):
                            specs.append(
                                (pss[b // 8][32 * j:32 * j + rj,
                                             (b % 8) * U:(b % 8 + 1) * U],
                                 xtt[:, b, 32 * j:32 * j + rj],
                                 k0b_sb[:, :], b // 8, j))
                    base_col = blk * 4 * T * 32
                    base_idx = blk * 4 * T * 8
                    for u0 in range(0, 4 * T, cfg.GU):
                        live = [t for t in range(u0, u0 + cfg.GU)
                                if meta.jmap[t] in groups]
                        if not live:
                            continue
                        nt = len(live)
                        it = ipool.tile([128, nt * 8], I16, tag="idx")
                        nc.sync.dma_start(
                            it[:, :],
                            idx_t.ap()[:, base_idx + live[0] * 8:
                                       base_idx + live[0] * 8 + nt * 8])
                        sl = spool.tile([128, nt * 32], BF16, tag="sel")
                        nc.sync.dma_start(
                            sl[:, :],
                            sel_t.ap()[:, base_col + live[0] * 32:
                                       base_col + live[0] * 32 + nt * 32])
                        gt = gpool.tile([128, nt, F], BF16, tag="g")
                        nc.gpsimd.dma_gather(
                            gt[:, :, :], y12_t.ap()[:, :], it[:, :],
                            num_idxs=nt * 128, num_idxs_reg=nt * 128,
                            elem_size=F, queue_num=gq % cfg.NQ)
                        gq += 1
                        for ti, t in enumerate(live):
                            j = meta.jmap[t]
                            for f in range(cfg.NCHUNK):
                                specs.append(
                                    (pss[f][32 * j:32 * (j + 1), :],
                                     sl[:, ti * 32:(ti + 1) * 32],
                                     gt[:, ti, f * 512:(f + 1) * 512], f, j))

                    first = {}
                    last = {}
                    for i, sp in enumerate(specs):
                        first.setdefault((sp[3], sp[4]), i)
                        last[(sp[3], sp[4])] = i
                    prev_mm = None
                    for i, (out_ap, lhsT, rhs, f, j) in enumerate(specs):
                        mm = nc.tensor.matmul(
                            out_ap, lhsT, rhs,
                            start=(first[(f, j)] == i),
                            stop=(last[(f, j)] == i),
                            tile_position=(0, 32 * j),
                            skip_group_check=True)
                        if prev_mm is not None:
                            add_dep_helper(mm.ins, prev_mm.ins, sync=False,
                                           reason="psum accumulation order")
                        prev_mm = mm

                    ot = opool.tile([128, F], F32, tag="ot")
                    for f in range(cfg.NCHUNK):
                        nc.any.tensor_copy(ot[:nn, f * 512:(f + 1) * 512],
                                           pss[f][:nn, :])
                    for b in range(B):
                        nc.scalar.dma_start(out_t.ap()[b, n0:n0 + nn, :],
                                            ot[:nn, b * U:(b + 1) * U])
    return nc


def run(cfg: Cfg, inputs, trace=False, **spmd_kwargs):
    supports = [(np.asarray(inputs["sup0_rows"]), np.asarray(inputs["sup0_cols"]),
                 np.asarray(inputs["sup0_vals"], np.float32)),
                (np.asarray(inputs["sup1_rows"]), np.asarray(inputs["sup1_cols"]),
                 np.asarray(inputs["sup1_vals"], np.float32))]
    meta, idx_by_core, sel_by_core = preprocess_edges(cfg, supports)
    kc12, k0b = prep_weights(cfg, np.asarray(inputs["kernel"], np.float32),
                             np.asarray(inputs["bias"], np.float32))
    xt_full = prep_x(cfg, np.asarray(inputs["x"], np.float32))
    nc = build_nc(cfg, meta)
    nc.compile()
    in_maps = []
    for c in range(cfg.n_cores):
        in_maps.append({
            "xt": xt_full,
            "xo": prep_x_core(cfg, xt_full, c),
            "kc12": kc12,
            "k0b": k0b,
            "idx16": idx_by_core[c],
            "sel": sel_by_core[c],
        })

    from concourse.bass_utils import run_bass_kernel_spmd
    res = run_bass_kernel_spmd(nc, in_maps, core_ids=list(range(cfg.n_cores)),
                               trace=trace, **spmd_kwargs)
    out = np.concatenate([res.results[c]["out"] for c in range(cfg.n_cores)],
                         axis=1)
    return out, res


def kernel(**inputs) -> np.ndarray:
    """Full MGCN layer: takes the unsharded inputs of reference.setup_inputs()
    and returns the full [B, N, UNITS] float32 output."""
    out, _ = run(Cfg(), inputs, trace=False)
    return np.asarray(out, np.float32)


# revision 22
# speedup vs baseline: 1.7354x; 1.0636x over previous
"""MGCN (multi-graph GCN layer) Trainium2 kernel.

Math: with K0/K1/K2 = kernel rows de-interleaved (kernel[d*3+mx, u]),
  out[b] = X[b] @ K0 + bias + A0 @ (X[b] @ K1) + A1 @ (X[b] @ K2)
because the SpMM (over nodes) commutes with the per-feature projection.

Sharding: node-parallel for the SpMM. Core c owns output rows
[c*1250, (c+1)*1250) for ALL 64 batches. Every core redundantly computes the
full projections Y1 = X@K1, Y2 = X@K2 (cheap in bf16 on the PE) and writes
them row-interleaved into a local HBM scratch Y12[2n+s] = Ys[n] of shape
[2N, B*U] bf16, so the SpMM gather needs no cross-core traffic.

Stage 2: per output block of 128 rows, the edges of both supports (grouped by
32-row subgroup, sorted, padded to 128-edge tiles; padded to a uniform tile
count so all 8 cores run one identical SPMD program) are gathered with
dma_gather as full 8KB bf16 rows of Y12 (idx = 2*col + support), and the PE
accumulates segment sums via selector matmuls
  psum_f[32j:32j+32, :] += SelT[128e, 32r].T @ G[128e, f*512:(f+1)*512]
into 8 chunk-PSUM banks (one per group of 8 batches). The X@K0+bias term is
added by small per-(j, batch) matmuls from a per-core xt slice, then each
bank is copied out and written strided into the [B, N, U] output.

The single dma_gather descriptor per edge moves 8KB, which keeps the GpSimd
(SWDGE descriptor generation) cost ~8x below the HBM/DMA time — the kernel is
HBM-bound on the irreducible gather traffic.
"""

import math
from dataclasses import dataclass, field

import numpy as np
import ml_dtypes

import concourse.bass as bass
import concourse.bacc as bacc
import concourse.mybir as mybir
from concourse.tile import TileContext, add_dep_helper

F32 = mybir.dt.float32
BF16 = mybir.dt.bfloat16
I16 = mybir.dt.int16


@dataclass
class Cfg:
    B: int = 64          # total batches
    N: int = 10000       # nodes
    D: int = 64          # input features
    U: int = 64          # units
    n_cores: int = 8
    GU: int = 2          # gather-unit size in 128-edge tiles (elem = 8KB);
                         # small units keep 4 col-groups' tiles live at once
    CHUNK: int = 256     # stage-1 node chunk (multiple of 128)
    DMA_SCRATCH: int = 16384
    NQ: int = 4          # SWDGE queues; gathers round-robin across them

    @property
    def F(self):         # full feature width B*U
        return self.B * self.U

    @property
    def NPC(self):       # nodes (output rows) per core
        return self.N // self.n_cores

    @property
    def KD(self):        # contraction dim incl. ones row
        return self.D + 1

    @property
    def NT(self):        # stage-1 node tiles of 128 (full projection)
        return (self.N + 127) // 128

    @property
    def NBLK(self):      # per-core output blocks of 128 rows
        return (self.NPC + 127) // 128

    @property
    def NCHUNK(self):    # 512-col feature chunks
        return self.F // 512


@dataclass
class EdgeMeta:
    T: list                         # [blk][j] -> tile count (same all cores)
    idx_off: list                   # [blk][j] -> column offset into idx_all/8
    sel_off: list                   # [blk][j] -> column offset into sel_all/32
    idx_shape: tuple
    sel_shape: tuple


def preprocess_edges(cfg: Cfg, supports):
    """Build per-core idx/sel arrays with a uniform SPMD structure.

    Returns (meta, idx_by_core [n_cores, 128, W_i], sel_by_core).
    Edge (r, c, v) of support s gathers Y12 row 2c+s; it lands in core
    r // NPC, block (r % NPC) // 128, subgroup ((r % NPC) % 128) // 32.
    """
    N, NPC = cfg.N, cfg.NPC
    n_groups_rows = []  # per (core, blk, j): (idx_list, val, lr)
    groups = {}
    for s, (rows, cols, vals) in enumerate(supports):
        rows = np.asarray(rows)
        cols = np.asarray(cols)
        vals = np.asarray(vals, np.float32)
        order = np.argsort(rows, kind="stable")
        r, c, v = rows[order], cols[order], vals[order]
        core = r // NPC
        rr = r % NPC
        blk = rr // 128
        j = (rr % 128) // 32
        lr = rr % 32
        gidx = 2 * c + s
        key = np.stack([core, blk, j])
        for cc in range(cfg.n_cores):
            m0 = core == cc
            for bb in range(cfg.NBLK):
                m1 = m0 & (blk == bb)
                for jj in range(4):
                    m = m1 & (j == jj)
                    if not m.any():
                        continue
                    g = groups.setdefault((cc, bb, jj), [[], [], []])
                    g[0].append(gidx[m])
                    g[1].append(v[m])
                    g[2].append(lr[m])

    # per-(blk, j) tile count: max over cores (keeps SPMD, minimizes padding)
    def glen(key):
        g = groups.get(key)
        return sum(len(a) for a in g[0]) if g else 0

    T = [[0] * 4 for _ in range(cfg.NBLK)]
    for bb in range(cfg.NBLK):
        for jj in range(4):
            mx = max(glen((cc, bb, jj)) for cc in range(cfg.n_cores))
            T[bb][jj] = (mx + 127) // 128

    idx_off = [[0] * 4 for _ in range(cfg.NBLK)]
    sel_off = [[0] * 4 for _ in range(cfg.NBLK)]
    io = so = 0
    for bb in range(cfg.NBLK):
        for jj in range(4):
            idx_off[bb][jj] = io
            sel_off[bb][jj] = so
            io += T[bb][jj] * 8
            so += T[bb][jj] * 32

    idx_by_core, sel_by_core = [], []
    for cc in range(cfg.n_cores):
        idx_cols, sel_cols = [], []
        for bb in range(cfg.NBLK):
            for jj in range(4):
                Tt = T[bb][jj]
                if Tt == 0:
                    continue
                g = groups.get((cc, bb, jj))
                if g is None:
                    gi = np.zeros(0, np.int64)
                    gv = np.zeros(0, np.float32)
                    gl = np.zeros(0, np.int64)
                else:
                    gi = np.concatenate(g[0])
                    gv = np.concatenate(g[1])
                    gl = np.concatenate(g[2])
                pad = Tt * 128 - len(gi)
                gi = np.concatenate([gi, np.zeros(pad, np.int64)])
                gv = np.concatenate([gv, np.zeros(pad, np.float32)])
                gl = np.concatenate([gl, np.zeros(pad, np.int64)])
                # idx wrap: index i -> [i % 16, i // 16], replicated x8
                wrapped = gi.astype(np.int16).reshape(Tt * 8, 16).T
                idx_cols.append(np.tile(wrapped, (8, 1)))
                sel = np.zeros((128, Tt, 32), np.float32)
                lane = np.arange(Tt * 128) % 128
                tt = np.arange(Tt * 128) // 128
                sel[lane, tt, gl] = gv
                sel_cols.append(sel.reshape(128, Tt * 32)
                                .astype(ml_dtypes.bfloat16))
        idx_by_core.append(np.ascontiguousarray(np.concatenate(idx_cols, axis=1)))
        sel_by_core.append(np.ascontiguousarray(np.concatenate(sel_cols, axis=1)))

    meta = EdgeMeta(T=T, idx_off=idx_off, sel_off=sel_off,
                    idx_shape=idx_by_core[0].shape,
                    sel_shape=sel_by_core[0].shape)
    return meta, idx_by_core, sel_by_core


def prep_weights(cfg: Cfg, kernel, bias):
    K = kernel.reshape(cfg.D, 3, cfg.U)
    kc12 = np.zeros((cfg.KD, 2 * cfg.U), np.float32)
    kc12[:cfg.D, :cfg.U] = K[:, 1]
    kc12[:cfg.D, cfg.U:] = K[:, 2]
    k0b = np.zeros((cfg.KD, cfg.U), np.float32)
    k0b[:cfg.D] = K[:, 0]
    k0b[cfg.D] = bias
    return (kc12.astype(ml_dtypes.bfloat16), k0b.astype(ml_dtypes.bfloat16))


def prep_x(cfg: Cfg, x):
    """x [B, N, D] f32 -> xt_full [KD, B, N] bf16 (d-major, ones row)."""
    xt = np.empty((cfg.KD, cfg.B, cfg.N), np.float32)
    xt[:cfg.D] = x.transpose(2, 0, 1)
    xt[cfg.D] = 1.0
    return np.ascontiguousarray(xt.astype(ml_dtypes.bfloat16))


def prep_x_core(cfg: Cfg, xt_full, core):
    """xt_own [KD, B, NPC] bf16 slice for the X@K0+bias term."""
    sl = xt_full[:, :, core * cfg.NPC:(core + 1) * cfg.NPC]
    return np.ascontiguousarray(sl)


def build_nc(cfg: Cfg, meta: EdgeMeta):
    nc = bacc.Bacc("TRN2", num_devices=cfg.n_cores,
                   dynamic_dma_scratch_size=cfg.DMA_SCRATCH,
                   num_swdge_queues=cfg.NQ)
    KD, F, U, N, B = cfg.KD, cfg.F, cfg.U, cfg.N, cfg.B
    NPC = cfg.NPC

    xt_t = nc.dram_tensor("xt", [KD, B, N], BF16, kind="ExternalInput")
    xo_t = nc.dram_tensor("xo", [KD, B, NPC], BF16, kind="ExternalInput")
    kc12_t = nc.dram_tensor("kc12", [KD, 2 * U], BF16, kind="ExternalInput")
    k0b_t = nc.dram_tensor("k0b", [KD, U], BF16, kind="ExternalInput")
    idx_t = nc.dram_tensor("idx16", list(meta.idx_shape), I16,
                           kind="ExternalInput")
    sel_t = nc.dram_tensor("sel", list(meta.sel_shape), BF16,
                           kind="ExternalInput")
    y12_t = nc.dram_tensor("y12", [2 * N, F], BF16, kind="Internal")
    out_t = nc.dram_tensor("out", [B, NPC, U], F32, kind="ExternalOutput")

    with TileContext(nc) as tc:
        with tc.tile_pool(name="kpool", bufs=1) as kpool:
            kc_sb = kpool.tile([KD, 2 * U], BF16, tag="kc")
            nc.sync.dma_start(kc_sb[:, :], kc12_t.ap()[:, :])
            k0b_sb = kpool.tile([KD, U], BF16, tag="k0b")
            nc.sync.dma_start(k0b_sb[:, :], k0b_t.ap()[:, :])

            # ---- Stage 1: full projection Y12[2n+s] = (X @ K_{s+1})[n] ----
            with tc.tile_pool(name="xc", bufs=2) as xcpool, \
                 tc.tile_pool(name="st1", bufs=3) as stpool, \
                 tc.tile_pool(name="ps1", bufs=4, space="PSUM") as ps1pool:
                for c0 in range(0, N, cfg.CHUNK):
                    cw = min(cfg.CHUNK, N - c0)
                    xc = xcpool.tile([KD, B, cw], BF16, tag="xc")
                    nc.sync.dma_start(xc[:, :, :], xt_t.ap()[:, :, c0:c0 + cw])
                    for t0 in range(0, cw, 128):
                        nn = min(128, cw - t0)
                        st = stpool.tile([128, 2, F], BF16, tag="st")
                        for b8 in range(B // 8):
                            pp = ps1pool.tile([128, 8, 2 * U], F32, tag="pp")
                            for b2 in range(8):
                                b = b8 * 8 + b2
                                # the tile spans 2 PSUM banks; start clears
                                # one 2KB bank region, so restart per bank
                                nc.tensor.matmul(pp[:nn, b2, :],
                                                 xc[:, b, t0:t0 + nn],
                                                 kc_sb[:, :],
                                                 start=(b2 % 4 == 0),
                                                 stop=(b2 % 4 == 3),
                                                 skip_group_check=True)
                            # pp layout [n, b2, (s u)] -> st [n, s, (b2 u)]
                            nc.any.tensor_copy(
                                st[:nn, :, b8 * 512:b8 * 512 + 512]
                                .rearrange("p s (b2 u) -> p b2 s u", b2=8),
                                pp[:nn, :, :].rearrange(
                                    "p b2 (s u) -> p b2 s u", s=2))
                        n0 = c0 + t0
                        y12v = y12_t.ap().rearrange("(n s) f -> n s f", s=2)
                        nc.sync.dma_start(y12v[n0:n0 + nn, 0, :], st[:nn, 0, :])
                        nc.sync.dma_start(y12v[n0:n0 + nn, 1, :], st[:nn, 1, :])

            tc.strict_bb_all_engine_barrier()

            # ---- Stage 2: SpMM + X@K0 + bias, per 128-row block ----
            with tc.tile_pool(name="gp", bufs=6) as gpool, \
                 tc.tile_pool(name="ip", bufs=8) as ipool, \
                 tc.tile_pool(name="sp", bufs=8) as spool, \
                 tc.tile_pool(name="xb", bufs=2) as xbpool, \
                 tc.tile_pool(name="op", bufs=2) as opool, \
                 tc.tile_pool(name="ps2", bufs=1, space="PSUM") as ps2pool:
                gq = 0
                for blk in range(cfg.NBLK):
                    n0 = blk * 128
                    nn = min(128, NPC - n0)
                    groups = [j for j in range(4) if 32 * j < nn]
                    pss = [ps2pool.tile([128, 512], F32, tag=f"ps{f}",
                                        name=f"ps{f}")
                           for f in range(cfg.NCHUNK)]

                    xtt = xbpool.tile([KD, B, 128], BF16, tag="xtt")
                    nc.sync.dma_start(xtt[:, :, :nn],
                                      xo_t.ap()[:, :, n0:n0 + nn])

                    # (out, lhsT, rhs, chunk, j) — interleave across col
                    # groups j so adjacent PE matmuls target different 32-col
                    # strips of the array and execute concurrently.
                    y0_by_j = {j: [] for j in groups}
                    for j in groups:
                        rj = min(32, nn - 32 * j)
                        for b in range(B):
                            y0_by_j[j].append(
                                (pss[b // 8][32 * j:32 * j + rj,
                                             (b % 8) * U:(b % 8 + 1) * U],
                                 xtt[:, b, 32 * j:32 * j + rj],
                                 k0b_sb[:, :], b // 8, j))
                    # issue gathers in the SAME j-interleaved order the
                    # matmuls consume them — pool slots are granted in issue
                    # order, so per-j issue order would deadlock the chain
                    units_by_j = {j: list(range(0, meta.T[blk][j], cfg.GU))
                                  for j in groups}
                    sel_by_j = {j: [] for j in groups}
                    max_units = max((len(u) for u in units_by_j.values()),
                                    default=0)
                    for k in range(max_units):
                        for j in groups:
                            if k >= len(units_by_j[j]):
                                continue
                            u0 = units_by_j[j][k]
                            Tt = meta.T[blk][j]
                            nt = min(cfg.GU, Tt - u0)
                            io = (meta.idx_off[blk][j] + u0 * 8)
                            so = (meta.sel_off[blk][j] + u0 * 32)
                            it = ipool.tile([128, nt * 8], I16, tag="idx")
                            nc.sync.dma_start(it[:, :],
                                              idx_t.ap()[:, io:io + nt * 8])
                            sl = spool.tile([128, nt * 32], BF16, tag="sel")
                            nc.sync.dma_start(sl[:, :],
                                              sel_t.ap()[:, so:so + nt * 32])
                            gt = gpool.tile([128, nt, F], BF16, tag="g")
                            nc.gpsimd.dma_gather(
                                gt[:, :, :], y12_t.ap()[:, :], it[:, :],
                                num_idxs=nt * 128, num_idxs_reg=nt * 128,
                                elem_size=F, queue_num=gq % cfg.NQ)
                            gq += 1
                            for ti in range(nt):
                                for f in range(cfg.NCHUNK):
                                    sel_by_j[j].append(
                                        (pss[f][32 * j:32 * (j + 1), :],
                                         sl[:, ti * 32:(ti + 1) * 32],
                                         gt[:, ti, f * 512:(f + 1) * 512],
                                         f, j))

                    def interleave(by_j):
                        out = []
                        idxs = {j: 0 for j in by_j}
                        while True:
                            emitted = False
                            for j in by_j:
                                if idxs[j] < len(by_j[j]):
                                    out.append(by_j[j][idxs[j]])
                                    idxs[j] += 1
                                    emitted = True
                            if not emitted:
                                return out

                    specs = interleave(y0_by_j) + interleave(sel_by_j)

                    first = {}
                    last = {}
                    for i, sp in enumerate(specs):
                        first.setdefault((sp[3], sp[4]), i)
                        last[(sp[3], sp[4])] = i
                    prev_mm = None
                    for i, (out_ap, lhsT, rhs, f, j) in enumerate(specs):
                        mm = nc.tensor.matmul(
                            out_ap, lhsT, rhs,
                            start=(first[(f, j)] == i),
                            stop=(last[(f, j)] == i),
                            tile_position=(0, 32 * j),
                            skip_group_check=True)
                        if prev_mm is not None:
                            add_dep_helper(mm.ins, prev_mm.ins, sync=False,
                                           reason="psum accumulation order")
                        prev_mm = mm

                    ot = opool.tile([128, F], F32, tag="ot")
                    for f in range(cfg.NCHUNK):
                        nc.any.tensor_copy(ot[:nn, f * 512:(f + 1) * 512],
                                           pss[f][:nn, :])
                    for b in range(B):
                        nc.scalar.dma_start(out_t.ap()[b, n0:n0 + nn, :],
                                            ot[:nn, b * U:(b + 1) * U])
    return nc


def run(cfg: Cfg, inputs, trace=False, **spmd_kwargs):
    supports = [(np.asarray(inputs["sup0_rows"]), np.asarray(inputs["sup0_cols"]),
                 np.asarray(inputs["sup0_vals"], np.float32)),
                (np.asarray(inputs["sup1_rows"]), np.asarray(inputs["sup1_cols"]),
                 np.asarray(inputs["sup1_vals"], np.float32))]
    meta, idx_by_core, sel_by_core = preprocess_edges(cfg, supports)
    kc12, k0b = prep_weights(cfg, np.asarray(inputs["kernel"], np.float32),
                             np.asarray(inputs["bias"], np.float32))
    xt_full = prep_x(cfg, np.asarray(inputs["x"], np.float32))
    nc = build_nc(cfg, meta)
    nc.compile()
    in_maps = []
    for c in range(cfg.n_cores):
        in_maps.append({
            "xt": xt_full,
            "xo": prep_x_core(cfg, xt_full, c),
            "kc12": kc12,
            "k0b": k0b,
            "idx16": idx_by_core[c],
            "sel": sel_by_core[c],
        })

    from concourse.bass_utils import run_bass_kernel_spmd
    res = run_bass_kernel_spmd(nc, in_maps, core_ids=list(range(cfg.n_cores)),
                               trace=trace, **spmd_kwargs)
    out = np.concatenate([res.results[c]["out"] for c in range(cfg.n_cores)],
                         axis=1)
    return out, res


def kernel(**inputs) -> np.ndarray:
    """Full MGCN layer: takes the unsharded inputs of reference.setup_inputs()
    and returns the full [B, N, UNITS] float32 output."""
    out, _ = run(Cfg(), inputs, trace=False)
    return np.asarray(out, np.float32)


# revision 24
# speedup vs baseline: 1.7608x; 1.0146x over previous
"""MGCN (multi-graph GCN layer) Trainium2 kernel.

Math: with K0/K1/K2 = kernel rows de-interleaved (kernel[d*3+mx, u]),
  out[b] = X[b] @ K0 + bias + A0 @ (X[b] @ K1) + A1 @ (X[b] @ K2)
because the SpMM (over nodes) commutes with the per-feature projection.

Sharding: node-parallel for the SpMM. Core c owns output rows
[c*1250, (c+1)*1250) for ALL 64 batches. Every core redundantly computes the
full projections Y1 = X@K1, Y2 = X@K2 (cheap in bf16 on the PE) and writes
them row-interleaved into a local HBM scratch Y12[2n+s] = Ys[n] of shape
[2N, B*U] bf16, so the SpMM gather needs no cross-core traffic.

Stage 2: per output block of 128 rows, the edges of both supports (grouped by
32-row subgroup, sorted, padded to 128-edge tiles; padded to a uniform tile
count so all 8 cores run one identical SPMD program) are gathered with
dma_gather as full 8KB bf16 rows of Y12 (idx = 2*col + support), and the PE
accumulates segment sums via selector matmuls
  psum_f[32j:32j+32, :] += SelT[128e, 32r].T @ G[128e, f*512:(f+1)*512]
into 8 chunk-PSUM banks (one per group of 8 batches). The X@K0+bias term is
added by small per-(j, batch) matmuls from a per-core xt slice, then each
bank is copied out and written strided into the [B, N, U] output.

The single dma_gather descriptor per edge moves 8KB, which keeps the GpSimd
(SWDGE descriptor generation) cost ~8x below the HBM/DMA time — the kernel is
HBM-bound on the irreducible gather traffic.
"""

import math
from dataclasses import dataclass, field

import numpy as np
import ml_dtypes

import concourse.bass as bass
import concourse.bacc as bacc
import concourse.mybir as mybir
from concourse.tile import TileContext, add_dep_helper

F32 = mybir.dt.float32
BF16 = mybir.dt.bfloat16
FP8 = mybir.dt.float8e4
I16 = mybir.dt.int16


@dataclass
class Cfg:
    B: int = 64          # total batches
    N: int = 10000       # nodes
    D: int = 64          # input features
    U: int = 64          # units
    n_cores: int = 8
    GU: int = 2          # gather-unit size in 128-edge tiles (elem = 8KB);
                         # small units keep 4 col-groups' tiles live at once
    CHUNK: int = 512     # stage-1 node chunk (multiple of 128)
    DMA_SCRATCH: int = 16384
    NQ: int = 4          # SWDGE queues; gathers round-robin across them
    FP8_GATHER: bool = False  # fp8e4m3 gather path: halves DMA but rel err ~3e-2 (too lossy)

    @property
    def GDT(self):       # gather-path dtype
        return FP8 if self.FP8_GATHER else BF16

    @property
    def F(self):         # full feature width B*U
        return self.B * self.U

    @property
    def NPC(self):       # nodes (output rows) per core
        return self.N // self.n_cores

    @property
    def KD(self):        # contraction dim incl. ones row
        return self.D + 1

    @property
    def NT(self):        # stage-1 node tiles of 128 (full projection)
        return (self.N + 127) // 128

    @property
    def NBLK(self):      # per-core output blocks of 128 rows
        return (self.NPC + 127) // 128

    @property
    def NCHUNK(self):    # 512-col feature chunks
        return self.F // 512


@dataclass
class EdgeMeta:
    T: list                         # [blk][j] -> tile count (same all cores)
    idx_off: list                   # [blk][j] -> column offset into idx_all/8
    sel_off: list                   # [blk][j] -> column offset into sel_all/32
    idx_shape: tuple
    sel_shape: tuple


def preprocess_edges(cfg: Cfg, supports):
    """Build per-core idx/sel arrays with a uniform SPMD structure.

    Returns (meta, idx_by_core [n_cores, 128, W_i], sel_by_core).
    Edge (r, c, v) of support s gathers Y12 row 2c+s; it lands in core
    r // NPC, block (r % NPC) // 128, subgroup ((r % NPC) % 128) // 32.
    """
    N, NPC = cfg.N, cfg.NPC
    n_groups_rows = []  # per (core, blk, j): (idx_list, val, lr)
    groups = {}
    for s, (rows, cols, vals) in enumerate(supports):
        rows = np.asarray(rows)
        cols = np.asarray(cols)
        vals = np.asarray(vals, np.float32)
        order = np.argsort(rows, kind="stable")
        r, c, v = rows[order], cols[order], vals[order]
        core = r // NPC
        rr = r % NPC
        blk = rr // 128
        j = (rr % 128) // 32
        lr = rr % 32
        gidx = 2 * c + s
        key = np.stack([core, blk, j])
        for cc in range(cfg.n_cores):
            m0 = core == cc
            for bb in range(cfg.NBLK):
                m1 = m0 & (blk == bb)
                for jj in range(4):
                    m = m1 & (j == jj)
                    if not m.any():
                        continue
                    g = groups.setdefault((cc, bb, jj), [[], [], []])
                    g[0].append(gidx[m])
                    g[1].append(v[m])
                    g[2].append(lr[m])

    # per-(blk, j) tile count: max over cores (keeps SPMD, minimizes padding)
    def glen(key):
        g = groups.get(key)
        return sum(len(a) for a in g[0]) if g else 0

    T = [[0] * 4 for _ in range(cfg.NBLK)]
    for bb in range(cfg.NBLK):
        for jj in range(4):
            mx = max(glen((cc, bb, jj)) for cc in range(cfg.n_cores))
            T[bb][jj] = (mx + 127) // 128

    idx_off = [[0] * 4 for _ in range(cfg.NBLK)]
    sel_off = [[0] * 4 for _ in range(cfg.NBLK)]
    io = so = 0
    for bb in range(cfg.NBLK):
        for jj in range(4):
            idx_off[bb][jj] = io
            sel_off[bb][jj] = so
            io += T[bb][jj] * 8
            so += T[bb][jj] * 32

    idx_by_core, sel_by_core = [], []
    for cc in range(cfg.n_cores):
        idx_cols, sel_cols = [], []
        for bb in range(cfg.NBLK):
            for jj in range(4):
                Tt = T[bb][jj]
                if Tt == 0:
                    continue
                g = groups.get((cc, bb, jj))
                if g is None:
                    gi = np.zeros(0, np.int64)
                    gv = np.zeros(0, np.float32)
                    gl = np.zeros(0, np.int64)
                else:
                    gi = np.concatenate(g[0])
                    gv = np.concatenate(g[1])
                    gl = np.concatenate(g[2])
                pad = Tt * 128 - len(gi)
                gi = np.concatenate([gi, np.zeros(pad, np.int64)])
                gv = np.concatenate([gv, np.zeros(pad, np.float32)])
                gl = np.concatenate([gl, np.zeros(pad, np.int64)])
                # idx wrap: index i -> [i % 16, i // 16], replicated x8
                wrapped = gi.astype(np.int16).reshape(Tt * 8, 16).T
                idx_cols.append(np.tile(wrapped, (8, 1)))
                sel = np.zeros((128, Tt, 32), np.float32)
                lane = np.arange(Tt * 128) % 128
                tt = np.arange(Tt * 128) // 128
                sel[lane, tt, gl] = gv
                gdt = (ml_dtypes.float8_e4m3 if cfg.FP8_GATHER
                       else ml_dtypes.bfloat16)
                sel_cols.append(sel.reshape(128, Tt * 32).astype(gdt))
        idx_by_core.append(np.ascontiguousarray(np.concatenate(idx_cols, axis=1)))
        sel_by_core.append(np.ascontiguousarray(np.concatenate(sel_cols, axis=1)))

    meta = EdgeMeta(T=T, idx_off=idx_off, sel_off=sel_off,
                    idx_shape=idx_by_core[0].shape,
                    sel_shape=sel_by_core[0].shape)
    return meta, idx_by_core, sel_by_core


def prep_weights(cfg: Cfg, kernel, bias):
    K = kernel.reshape(cfg.D, 3, cfg.U)
    kc12 = np.zeros((cfg.KD, 2 * cfg.U), np.float32)
    kc12[:cfg.D, :cfg.U] = K[:, 1]
    kc12[:cfg.D, cfg.U:] = K[:, 2]
    k0b = np.zeros((cfg.KD, cfg.U), np.float32)
    k0b[:cfg.D] = K[:, 0]
    k0b[cfg.D] = bias
    return (kc12.astype(ml_dtypes.bfloat16), k0b.astype(ml_dtypes.bfloat16))


def prep_x(cfg: Cfg, x):
    """x [B, N, D] f32 -> xt_full [KD, B, N] bf16 (d-major, ones row)."""
    xt = np.empty((cfg.KD, cfg.B, cfg.N), np.float32)
    xt[:cfg.D] = x.transpose(2, 0, 1)
    xt[cfg.D] = 1.0
    return np.ascontiguousarray(xt.astype(ml_dtypes.bfloat16))


def prep_x_core(cfg: Cfg, xt_full, core):
    """xt_own [KD, B, NPC] bf16 slice for the X@K0+bias term."""
    sl = xt_full[:, :, core * cfg.NPC:(core + 1) * cfg.NPC]
    return np.ascontiguousarray(sl)


def build_nc(cfg: Cfg, meta: EdgeMeta):
    nc = bacc.Bacc("TRN2", num_devices=cfg.n_cores,
                   dynamic_dma_scratch_size=cfg.DMA_SCRATCH,
                   num_swdge_queues=cfg.NQ)
    KD, F, U, N, B = cfg.KD, cfg.F, cfg.U, cfg.N, cfg.B
    NPC = cfg.NPC

    xt_t = nc.dram_tensor("xt", [KD, B, N], BF16, kind="ExternalInput")
    xo_t = nc.dram_tensor("xo", [KD, B, NPC], BF16, kind="ExternalInput")
    kc12_t = nc.dram_tensor("kc12", [KD, 2 * U], BF16, kind="ExternalInput")
    k0b_t = nc.dram_tensor("k0b", [KD, U], BF16, kind="ExternalInput")
    idx_t = nc.dram_tensor("idx16", list(meta.idx_shape), I16,
                           kind="ExternalInput")
    GDT = cfg.GDT
    sel_t = nc.dram_tensor("sel", list(meta.sel_shape), GDT,
                           kind="ExternalInput")
    y12_t = nc.dram_tensor("y12", [2 * N, F], GDT, kind="Internal")
    out_t = nc.dram_tensor("out", [B, NPC, U], F32, kind="ExternalOutput")

    with TileContext(nc) as tc:
        with tc.tile_pool(name="kpool", bufs=1) as kpool:
            kc_sb = kpool.tile([KD, 2 * U], BF16, tag="kc")
            nc.sync.dma_start(kc_sb[:, :], kc12_t.ap()[:, :])
            k0b_sb = kpool.tile([KD, U], BF16, tag="k0b")
            nc.sync.dma_start(k0b_sb[:, :], k0b_t.ap()[:, :])

            # ---- Stage 1: full projection Y12[2n+s] = (X @ K_{s+1})[n] ----
            y12_writes = []
            with tc.tile_pool(name="xc", bufs=2) as xcpool, \
                 tc.tile_pool(name="st1", bufs=3) as stpool, \
                 tc.tile_pool(name="ps1", bufs=4, space="PSUM") as ps1pool:
                for c0 in range(0, N, cfg.CHUNK):
                    cw = min(cfg.CHUNK, N - c0)
                    xc = xcpool.tile([KD, B, cw], BF16, tag="xc")
                    nc.sync.dma_start(xc[:, :, :], xt_t.ap()[:, :, c0:c0 + cw])
                    for t0 in range(0, cw, 128):
                        nn = min(128, cw - t0)
                        st = stpool.tile([128, 2, F], GDT, tag="st")
                        for b8 in range(B // 8):
                            pp = ps1pool.tile([128, 8, 2 * U], F32, tag="pp")
                            for b2 in range(8):
                                b = b8 * 8 + b2
                                # the tile spans 2 PSUM banks; start clears
                                # one 2KB bank region, so restart per bank
                                nc.tensor.matmul(pp[:nn, b2, :],
                                                 xc[:, b, t0:t0 + nn],
                                                 kc_sb[:, :],
                                                 start=(b2 % 4 == 0),
                                                 stop=(b2 % 4 == 3),
                                                 skip_group_check=True)
                            # pp layout [n, b2, (s u)] -> st [n, s, (b2 u)]
                            nc.any.tensor_copy(
                                st[:nn, :, b8 * 512:b8 * 512 + 512]
                                .rearrange("p s (b2 u) -> p b2 s u", b2=8),
                                pp[:nn, :, :].rearrange(
                                    "p b2 (s u) -> p b2 s u", s=2))
                        n0 = c0 + t0
                        y12v = y12_t.ap().rearrange("(n s) f -> n s f", s=2)
                        y12_writes.append(nc.sync.dma_start(
                            y12v[n0:n0 + nn, 0, :], st[:nn, 0, :]))
                        y12_writes.append(nc.sync.dma_start(
                            y12v[n0:n0 + nn, 1, :], st[:nn, 1, :]))

            # Gate ONLY the gathers on stage 1 (Tile does not track DRAM RAW
            # deps): a nop that depends on every Y12 write, which every
            # gather then depends on. Leaves Y0 matmuls and sel/idx/xtt
            # prefetch free to overlap stage 1.
            y12_done = nc.sync.nop()
            for w in y12_writes:
                add_dep_helper(y12_done.ins, w.ins, sync=True,
                               reason="y12 complete")

            # ---- Stage 2: SpMM + X@K0 + bias, per 128-row block ----
            with tc.tile_pool(name="gp", bufs=6) as gpool, \
                 tc.tile_pool(name="ip", bufs=8) as ipool, \
                 tc.tile_pool(name="sp", bufs=8) as spool, \
                 tc.tile_pool(name="xb", bufs=2) as xbpool, \
                 tc.tile_pool(name="op", bufs=2) as opool, \
                 tc.tile_pool(name="ps2", bufs=1, space="PSUM") as ps2pool:
                gq = 0
                for blk in range(cfg.NBLK):
                    n0 = blk * 128
                    nn = min(128, NPC - n0)
                    groups = [j for j in range(4) if 32 * j < nn]
                    pss = [ps2pool.tile([128, 512], F32, tag=f"ps{f}",
                                        name=f"ps{f}")
                           for f in range(cfg.NCHUNK)]

                    xtt = xbpool.tile([KD, B, 128], BF16, tag="xtt")
                    nc.sync.dma_start(xtt[:, :, :nn],
                                      xo_t.ap()[:, :, n0:n0 + nn])

                    # (out, lhsT, rhs, chunk, j) — interleave across col
                    # groups j so adjacent PE matmuls target different 32-col
                    # strips of the array and execute concurrently.
                    y0_by_j = {j: [] for j in groups}
                    for j in groups:
                        rj = min(32, nn - 32 * j)
                        for b in range(B):
                            y0_by_j[j].append(
                                (pss[b // 8][32 * j:32 * j + rj,
                                             (b % 8) * U:(b % 8 + 1) * U],
                                 xtt[:, b, 32 * j:32 * j + rj],
                                 k0b_sb[:, :], b // 8, j))
                    # issue gathers in the SAME j-interleaved order the
                    # matmuls consume them — pool slots are granted in issue
                    # order, so per-j issue order would deadlock the chain
                    units_by_j = {j: list(range(0, meta.T[blk][j], cfg.GU))
                                  for j in groups}
                    sel_by_j = {j: [] for j in groups}
                    max_units = max((len(u) for u in units_by_j.values()),
                                    default=0)
                    for k in range(max_units):
                        for j in groups:
                            if k >= len(units_by_j[j]):
                                continue
                            u0 = units_by_j[j][k]
                            Tt = meta.T[blk][j]
                            nt = min(cfg.GU, Tt - u0)
                            io = (meta.idx_off[blk][j] + u0 * 8)
                            so = (meta.sel_off[blk][j] + u0 * 32)
                            it = ipool.tile([128, nt * 8], I16, tag="idx")
                            nc.sync.dma_start(it[:, :],
                                              idx_t.ap()[:, io:io + nt * 8])
                            sl = spool.tile([128, nt * 32], GDT, tag="sel")
                            nc.sync.dma_start(sl[:, :],
                                              sel_t.ap()[:, so:so + nt * 32])
                            gt = gpool.tile([128, nt, F], GDT, tag="g")
                            gi_ = nc.gpsimd.dma_gather(
                                gt[:, :, :], y12_t.ap()[:, :], it[:, :],
                                num_idxs=nt * 128, num_idxs_reg=nt * 128,
                                elem_size=F, queue_num=gq % cfg.NQ)
                            add_dep_helper(gi_.ins, y12_done.ins, sync=True,
                                           reason="gather after y12")
                            gq += 1
                            for ti in range(nt):
                                for f in range(cfg.NCHUNK):
                                    sel_by_j[j].append(
                                        (pss[f][32 * j:32 * (j + 1), :],
                                         sl[:, ti * 32:(ti + 1) * 32],
                                         gt[:, ti, f * 512:(f + 1) * 512],
                                         f, j))

                    def interleave(by_j):
                        out = []
                        idxs = {j: 0 for j in by_j}
                        while True:
                            emitted = False
                            for j in by_j:
                                if idxs[j] < len(by_j[j]):
                                    out.append(by_j[j][idxs[j]])
                                    idxs[j] += 1
                                    emitted = True
                            if not emitted:
                                return out

                    specs = interleave(y0_by_j) + interleave(sel_by_j)

                    first = {}
                    last = {}
                    for i, sp in enumerate(specs):
                        first.setdefault((sp[3], sp[4]), i)
                        last[(sp[3], sp[4])] = i
                    prev_mm = None
                    for i, (out_ap, lhsT, rhs, f, j) in enumerate(specs):
                        mm = nc.tensor.matmul(
                            out_ap, lhsT, rhs,
                            start=(first[(f, j)] == i),
                            stop=(last[(f, j)] == i),
                            tile_position=(0, 32 * j),
                            skip_group_check=True)
                        if prev_mm is not None:
                            add_dep_helper(mm.ins, prev_mm.ins, sync=False,
                                           reason="psum accumulation order")
                        prev_mm = mm

                    ot = opool.tile([128, F], F32, tag="ot")
                    for f in range(cfg.NCHUNK):
                        nc.any.tensor_copy(ot[:nn, f * 512:(f + 1) * 512],
                                           pss[f][:nn, :])
                    for b in range(B):
                        nc.scalar.dma_start(out_t.ap()[b, n0:n0 + nn, :],
                                            ot[:nn, b * U:(b + 1) * U])
    return nc


def run(cfg: Cfg, inputs, trace=False, **spmd_kwargs):
    supports = [(np.asarray(inputs["sup0_rows"]), np.asarray(inputs["sup0_cols"]),
                 np.asarray(inputs["sup0_vals"], np.float32)),
                (np.asarray(inputs["sup1_rows"]), np.asarray(inputs["sup1_cols"]),
                 np.asarray(inputs["sup1_vals"], np.float32))]
    meta, idx_by_core, sel_by_core = preprocess_edges(cfg, supports)
    kc12, k0b = prep_weights(cfg, np.asarray(inputs["kernel"], np.float32),
                             np.asarray(inputs["bias"], np.float32))
    xt_full = prep_x(cfg, np.asarray(inputs["x"], np.float32))
    nc = build_nc(cfg, meta)
    nc.compile()
    in_maps = []
    for c in range(cfg.n_cores):
        in_maps.append({
            "xt": xt_full,
            "xo": prep_x_core(cfg, xt_full, c),
            "kc12": kc12,
            "k0b": k0b,
            "idx16": idx_by_core[c],
            "sel": sel_by_core[c],
        })

    from concourse.bass_utils import run_bass_kernel_spmd
    res = run_bass_kernel_spmd(nc, in_maps, core_ids=list(range(cfg.n_cores)),
                               trace=trace, **spmd_kwargs)
    out = np.concatenate([res.results[c]["out"] for c in range(cfg.n_cores)],
                         axis=1)
    return out, res


def kernel(**inputs) -> np.ndarray:
    """Full MGCN layer: takes the unsharded inputs of reference.setup_inputs()
    and returns the full [B, N, UNITS] float32 output."""
    out, _ = run(Cfg(), inputs, trace=False)
    return np.asarray(out, np.float32)
